# revision 3
# baseline (speedup 1.0000x reference)
"""Sharded SLAY sampled-softmax loss on 8 NeuronCores.

Sharding (per spec hint): label axis (32768) is split across the 8 cores for
the dominant phi_W/Z computation; the batch (512) is data-parallel for the
query gather/mean, phi_query, and positive-label terms. The 4096-dim partial
Z vectors and per-shard partial losses are combined with psum collectives.

Key algebraic point: Z_approx_vec = sum_n phi_W[n] never materializes the
[32768, 4096] feature matrix — for each (r, h) it is the matmul
poly_feat[:, h, :].T @ prf_feat[r, :, h, :] contracted over the label axis.
"""
import numpy as np
import jax
import jax.numpy as jnp
from functools import partial

# Hyperparams (fixed by the problem)
H, D, P, M, R = 4, 128, 16, 32, 2
EPS = 1e-6
C = 2.0 + EPS
_nodes, _weights = np.polynomial.laguerre.laggauss(R)
QUAD_NODES = np.asarray(_nodes, np.float32) / C
QUAD_WEIGHTS = np.asarray(_weights, np.float32) / C

VOCAB, NUM_LABELS, EMBED_DIM = 100000, 32768, 512
B, S, K = 512, 64, 5
N_CORES = 8
L_SHARD = NUM_LABELS // N_CORES   # 4096 labels per core
B_SHARD = B // N_CORES            # 64 queries per core


def _features_factors(x):
    """Normalized per-head poly and prf factors for rows of x [N, E].

    Returns poly [N, H, P] and prf [R, N, H, M]; phi would be their
    per-(r,h) outer product flattened — we keep factors when possible.
    """
    n = x.shape[0]
    xr = x.reshape(n, H, D)
    xn = xr / jnp.clip(jnp.linalg.norm(xr, axis=-1, keepdims=True), 1e-6)
    return xn


def _poly_prf(xn, omega, anchors):
    poly = jnp.einsum('nhd,pd->nhp', xn, anchors) ** 2 / jnp.sqrt(float(P))
    proj = jnp.einsum('nhd,rhdm->rnhm', xn, omega)
    s = jnp.asarray(QUAD_NODES).reshape(R, 1, 1, 1)
    sqrt_2s = jnp.sqrt(2.0 * jnp.clip(s, 0))
    prf = jnp.exp(jnp.clip(proj * sqrt_2s - s, -10.0, 10.0)) / jnp.sqrt(float(M))
    prf = prf * jnp.sqrt(jnp.clip(jnp.asarray(QUAD_WEIGHTS).reshape(R, 1, 1, 1), 0))
    return poly, prf


def _phi_explicit(poly, prf):
    """fused[n, r, h, p, m] -> [N, R*H*P*M], matching reference ordering."""
    fused = jnp.einsum('nhp,rnhm->nrhpm', poly, prf)
    return fused.reshape(poly.shape[0], -1)


def _build_loss_fn():
    @partial(jax.pmap, axis_name='x',
             in_axes=(0, 0, 0, 0, 0, 0, 0, 0, 0),
             out_axes=None)
    def loss_fn(indices, mask, labels, label_mask, w_slice,
                w_all_t, embedding_table, omega, anchors):
        # ---- label shard: partial Z ----
        w_vecs = w_slice.T                                # [L_SHARD, E]
        xn_w = _features_factors(w_vecs)
        poly_w, prf_w = _poly_prf(xn_w, omega, anchors)
        z_part = jnp.einsum('nhp,rnhm->rhpm', poly_w, prf_w)
        z_vec = jax.lax.psum(z_part.reshape(-1), 'x')     # [4096]

        # ---- batch shard: queries ----
        embeds = embedding_table[indices]                 # [b, S, E]
        sum_embeds = jnp.sum(embeds * mask[:, :, None], axis=1)
        query = sum_embeds / jnp.clip(jnp.sum(mask, axis=1, keepdims=True), 1e-9)
        xn_q = _features_factors(query)
        poly_q, prf_q = _poly_prf(xn_q, omega, anchors)
        phi_q = _phi_explicit(poly_q, prf_q)              # [b, 4096]

        denom = phi_q @ z_vec + 1e-6
        log_z = jnp.log(denom)                            # [b]

        # ---- positives for this batch shard ----
        safe_labels = jnp.maximum(labels, 0)              # [b, K]
        w_pos = w_all_t[safe_labels.reshape(-1)]          # [b*K, E]
        xn_p = _features_factors(w_pos)
        poly_p, prf_p = _poly_prf(xn_p, omega, anchors)
        # factorized dot: nums[i] = sum_{r,h} (poly_p.poly_q)(prf_p.prf_q)
        b = labels.shape[0]
        poly_qr = jnp.repeat(poly_q, K, axis=0)           # [b*K, H, P]
        prf_qr = jnp.repeat(prf_q, K, axis=1)             # [R, b*K, H, M]
        a_dot = jnp.einsum('nhp,nhp->nh', poly_p, poly_qr)
        b_dot = jnp.einsum('rnhm,rnhm->rnh', prf_p, prf_qr)
        nums = jnp.sum(a_dot[None] * b_dot, axis=(0, 2)).reshape(b, K) + 1e-6
        log_probs = jnp.log(nums) - log_z[:, None]
        part_loss = -jnp.sum(log_probs * label_mask)
        return jax.lax.psum(part_loss, 'x') / B

    return loss_fn


_LOSS_FN = _build_loss_fn()


def kernel(indices, mask, labels, label_mask, embedding_table,
           classifier_kernel, omega, anchors):
    indices = np.asarray(indices).astype(np.int32)
    labels = np.asarray(labels).astype(np.int32)
    mask = np.asarray(mask, dtype=np.float32)
    label_mask = np.asarray(label_mask, dtype=np.float32)
    embedding_table = np.asarray(embedding_table, dtype=np.float32)
    classifier_kernel = np.asarray(classifier_kernel, dtype=np.float32)
    omega = np.asarray(omega, dtype=np.float32)
    anchors = np.asarray(anchors, dtype=np.float32)

    # shard batch-parallel tensors
    idx_sh = indices.reshape(N_CORES, B_SHARD, S)
    mask_sh = mask.reshape(N_CORES, B_SHARD, S)
    lab_sh = labels.reshape(N_CORES, B_SHARD, K)
    lmask_sh = label_mask.reshape(N_CORES, B_SHARD, K)
    # shard the label axis of the classifier
    w_sh = classifier_kernel.reshape(EMBED_DIM, N_CORES, L_SHARD).transpose(1, 0, 2)
    w_all_t = classifier_kernel.T  # [NUM_LABELS, E] replicated for positives

    def rep(a):
        return np.broadcast_to(a, (N_CORES,) + a.shape)

    out = _LOSS_FN(idx_sh, mask_sh, lab_sh, lmask_sh, w_sh,
                   rep(w_all_t), rep(embedding_table), rep(omega), rep(anchors))
    return np.asarray(out, dtype=np.float32)



# revision 4
# speedup vs baseline: 1.0485x; 1.0485x over previous
"""Sharded SLAY sampled-softmax loss on 8 NeuronCores — v2.

Changes vs baseline:
  * heavy einsums (label-shard phi_W factors, projections) run in bf16 with
    fp32 accumulation — PE runs bf16 at 4x the fp32 rate and the loss
    tolerance (2e-2) comfortably absorbs it;
  * only ONE collective (psum of the 4096-dim Z vector); per-core partial
    losses are returned to host and summed there (saves an all-reduce);
  * loss math kept in fp32.
"""
import numpy as np
import jax
import jax.numpy as jnp
from functools import partial

H, D, P, M, R = 4, 128, 16, 32, 2
EPS = 1e-6
C = 2.0 + EPS
_nodes, _weights = np.polynomial.laguerre.laggauss(R)
QUAD_NODES = np.asarray(_nodes, np.float32) / C
QUAD_WEIGHTS = np.asarray(_weights, np.float32) / C

VOCAB, NUM_LABELS, EMBED_DIM = 100000, 32768, 512
B, S, K = 512, 64, 5
N_CORES = 8
L_SHARD = NUM_LABELS // N_CORES
B_SHARD = B // N_CORES

BF = jnp.bfloat16
F32 = jnp.float32


def _normalize(x):
    n = x.shape[0]
    xr = x.reshape(n, H, D)
    ss = jnp.sum(xr * xr, axis=-1, keepdims=True)
    return xr * jax.lax.rsqrt(jnp.maximum(ss, 1e-12))


def _poly_prf(xn, omega, anchors, low_precision):
    if low_precision:
        poly_raw = jnp.einsum('nhd,pd->nhp', xn.astype(BF), anchors.astype(BF),
                              preferred_element_type=F32)
        proj = jnp.einsum('nhd,rhdm->rnhm', xn.astype(BF), omega.astype(BF),
                          preferred_element_type=F32)
    else:
        poly_raw = jnp.einsum('nhd,pd->nhp', xn, anchors)
        proj = jnp.einsum('nhd,rhdm->rnhm', xn, omega)
    poly = poly_raw ** 2 / jnp.sqrt(float(P))
    s = jnp.asarray(QUAD_NODES).reshape(R, 1, 1, 1)
    sqrt_2s = jnp.sqrt(2.0 * jnp.clip(s, 0))
    prf = jnp.exp(jnp.clip(proj * sqrt_2s - s, -10.0, 10.0)) / jnp.sqrt(float(M))
    prf = prf * jnp.sqrt(jnp.clip(jnp.asarray(QUAD_WEIGHTS).reshape(R, 1, 1, 1), 0))
    return poly, prf


def _build_loss_fn():
    @partial(jax.pmap, axis_name='x',
             in_axes=(0, 0, 0, 0, 0, 0, 0, 0, 0),
             out_axes=0)
    def loss_fn(indices, mask, labels, label_mask, w_slice,
                w_all_t, embedding_table, omega, anchors):
        # ---- label shard: partial Z (dominant cost, bf16 matmuls) ----
        w_vecs = w_slice.T                                # [L_SHARD, E]
        xn_w = _normalize(w_vecs)
        poly_w, prf_w = _poly_prf(xn_w, omega, anchors, low_precision=True)
        z_part = jnp.einsum('nhp,rnhm->rhpm',
                            poly_w.astype(BF), prf_w.astype(BF),
                            preferred_element_type=F32)
        z_vec = jax.lax.psum(z_part.reshape(-1), 'x')     # [4096]

        # ---- batch shard: queries ----
        embeds = embedding_table[indices]                 # [b, S, E]
        sum_embeds = jnp.sum(embeds * mask[:, :, None], axis=1)
        query = sum_embeds / jnp.clip(jnp.sum(mask, axis=1, keepdims=True), 1e-9)
        xn_q = _normalize(query)
        poly_q, prf_q = _poly_prf(xn_q, omega, anchors, low_precision=False)
        fused = jnp.einsum('nhp,rnhm->nrhpm', poly_q, prf_q)
        phi_q = fused.reshape(poly_q.shape[0], -1)        # [b, 4096]

        denom = phi_q @ z_vec + 1e-6
        log_z = jnp.log(denom)                            # [b]

        # ---- positives for this batch shard ----
        safe_labels = jnp.maximum(labels, 0)              # [b, K]
        w_pos = w_all_t[safe_labels.reshape(-1)]          # [b*K, E]
        xn_p = _normalize(w_pos)
        poly_p, prf_p = _poly_prf(xn_p, omega, anchors, low_precision=False)
        b = labels.shape[0]
        poly_qr = jnp.repeat(poly_q, K, axis=0)           # [b*K, H, P]
        prf_qr = jnp.repeat(prf_q, K, axis=1)             # [R, b*K, H, M]
        a_dot = jnp.einsum('nhp,nhp->nh', poly_p, poly_qr)
        b_dot = jnp.einsum('rnhm,rnhm->rnh', prf_p, prf_qr)
        nums = jnp.sum(a_dot[None] * b_dot, axis=(0, 2)).reshape(b, K) + 1e-6
        log_probs = jnp.log(nums) - log_z[:, None]
        return -jnp.sum(log_probs * label_mask)           # per-core partial

    return loss_fn


_LOSS_FN = _build_loss_fn()


def kernel(indices, mask, labels, label_mask, embedding_table,
           classifier_kernel, omega, anchors):
    indices = np.asarray(indices).astype(np.int32)
    labels = np.asarray(labels).astype(np.int32)
    mask = np.asarray(mask, dtype=np.float32)
    label_mask = np.asarray(label_mask, dtype=np.float32)
    embedding_table = np.asarray(embedding_table, dtype=np.float32)
    classifier_kernel = np.asarray(classifier_kernel, dtype=np.float32)
    omega = np.asarray(omega, dtype=np.float32)
    anchors = np.asarray(anchors, dtype=np.float32)

    idx_sh = indices.reshape(N_CORES, B_SHARD, S)
    mask_sh = mask.reshape(N_CORES, B_SHARD, S)
    lab_sh = labels.reshape(N_CORES, B_SHARD, K)
    lmask_sh = label_mask.reshape(N_CORES, B_SHARD, K)
    w_sh = classifier_kernel.reshape(EMBED_DIM, N_CORES, L_SHARD).transpose(1, 0, 2)
    w_all_t = classifier_kernel.T

    def rep(a):
        return np.broadcast_to(a, (N_CORES,) + a.shape)

    parts = _LOSS_FN(idx_sh, mask_sh, lab_sh, lmask_sh, w_sh,
                     rep(w_all_t), rep(embedding_table), rep(omega), rep(anchors))
    return np.float32(np.sum(np.asarray(parts, dtype=np.float64)) / B)
